# revision 1
# baseline (speedup 1.0000x reference)
"""Causal self-attention (B=2, T=2048, D=1024, H=16) on 8 Trainium2 cores.

Sharding: tensor-parallel — core c = (b, g) with b = c // 4 (batch) and
g = c % 4 (head-group of 4 heads / 256 of the 1024 QKV output dims).
Each core computes its head-group's Q/K/V projections, attention, and the
partial output projection (rows g*256:(g+1)*256 of Wo); the host sums the
4 partials per batch (tensor-parallel unshard).

On-chip formulation is fully transposed (scores kept as S^T[k, q]) so no
on-device transposes are needed: the host feeds x^T per batch, and
  Q^T = Wq_g^T · x^T   (lhsT = Wq_g, rhs = x^T)
  S^T = K^T_h^T · Q^T  (lhsT = K^T tile, rhs = Q^T; heads packed in
                        partition halves 0:64 / 64:128 of the dq tiles)
  O^T = V_aug^T · P^T  (lhsT = V with a ones column -> row 64 of the
                        PSUM output accumulates the softmax denominators)
Softmax skips the max-subtraction (scores are O(10) for this problem's
scaling; exp is computed in fp32 from PSUM). The additive mask is applied
exactly: diagonal 128x128 blocks are added via PE matmuls
(lhsT = mask block, rhs = I). Strictly-lower-triangular blocks need no add
and upper blocks are skipped entirely (their exp underflows to 0) — that
fast path is only used when the host verifies the mask has that structure;
otherwise a general variant adds the full mask^T to every score block.
"""

import numpy as np
import ml_dtypes

bf16 = ml_dtypes.bfloat16

B, T, D = 2, 2048, 1024
H, HD = 16, 64
NCORES = 8
GH = 4                  # heads per core
GD = GH * HD            # 256 per-core qkv dims
NT = T // 128           # 16 t-tiles
KD = D // 128           # 8 contraction tiles over D
NQC = T // 512          # 4 q-chunks
SCALE = HD ** -0.5

TRACE = False
LAST_RESULT = None
_cache = {}


def _build(causal):
    import concourse.mybir as mybir
    import concourse.tile as tile
    from concourse import bacc
    from concourse.bass import ds, ts

    f32 = mybir.dt.float32
    bfl = mybir.dt.bfloat16
    Exp = mybir.ActivationFunctionType.Exp
    Ident = mybir.ActivationFunctionType.Identity

    nc = bacc.Bacc("TRN2", target_bir_lowering=False, debug=False,
                   num_devices=NCORES)

    xT_d = nc.dram_tensor("xT", [D, T], bfl, kind="ExternalInput").ap()
    wq_d = nc.dram_tensor("wq", [D, GD], bfl, kind="ExternalInput").ap()
    wk_d = nc.dram_tensor("wk", [D, GD], bfl, kind="ExternalInput").ap()
    wv_d = nc.dram_tensor("wv", [D, GD], bfl, kind="ExternalInput").ap()
    wo_d = nc.dram_tensor("wo", [GD, D], bfl, kind="ExternalInput").ap()
    bq_d = nc.dram_tensor("bq", [128, 2], f32, kind="ExternalInput").ap()
    bk_d = nc.dram_tensor("bk", [128, 2], f32, kind="ExternalInput").ap()
    bv_d = nc.dram_tensor("bv", [1, GD], f32, kind="ExternalInput").ap()
    bo_d = nc.dram_tensor("bo", [1, D], f32, kind="ExternalInput").ap()
    id_d = nc.dram_tensor("ident", [128, 128], bfl, kind="ExternalInput").ap()
    of_d = nc.dram_tensor("onesf", [1, 128], f32, kind="ExternalInput").ap()
    if causal:
        md_d = nc.dram_tensor("maskdiag", [NT, 128, 128], bfl,
                              kind="ExternalInput").ap()
    else:
        mt_d = nc.dram_tensor("maskT", [T, T], bfl, kind="ExternalInput").ap()
    out_d = nc.dram_tensor("out", [T, D], f32, kind="ExternalOutput").ap()

    with tile.TileContext(nc) as tc:
        with tc.tile_pool(name="cp", bufs=1) as cp, \
             tc.tile_pool(name="pr", bufs=1) as pr, \
             tc.tile_pool(name="pp", bufs=4) as pp, \
             tc.tile_pool(name="rp", bufs=4) as rp, \
             tc.tile_pool(name="rbp", bufs=4) as rbp, \
             tc.tile_pool(name="obp", bufs=3) as obp, \
             tc.tile_pool(name="outp", bufs=3) as outp, \
             tc.tile_pool(name="mchp", bufs=2) as mchp, \
             tc.tile_pool(name="sp", bufs=6, space="PSUM") as sp, \
             tc.tile_pool(name="op", bufs=2, space="PSUM") as op, \
             tc.tile_pool(name="dr", bufs=4, space="DRAM") as dr:

            # ---- constant loads ----
            xT_sb = cp.tile([128, KD, T], bfl, tag="xt")
            nc.sync.dma_start(out=xT_sb, in_=xT_d.rearrange("(k p) t -> p k t", p=128))
            wq_sb = cp.tile([128, KD, GD], bfl, tag="wq")
            wk_sb = cp.tile([128, KD, GD], bfl, tag="wk")
            wv_sb = cp.tile([128, KD, GD], bfl, tag="wv")
            nc.sync.dma_start(out=wq_sb, in_=wq_d.rearrange("(k p) m -> p k m", p=128))
            nc.sync.dma_start(out=wk_sb, in_=wk_d.rearrange("(k p) m -> p k m", p=128))
            nc.sync.dma_start(out=wv_sb, in_=wv_d.rearrange("(k p) m -> p k m", p=128))
            wo_sb = cp.tile([128, 2, D], bfl, tag="wo")
            nc.sync.dma_start(out=wo_sb, in_=wo_d.rearrange("(m p) n -> p m n", p=128))
            bq_sb = cp.tile([128, 2], f32, tag="bq")
            bk_sb = cp.tile([128, 2], f32, tag="bk")
            nc.sync.dma_start(out=bq_sb, in_=bq_d)
            nc.sync.dma_start(out=bk_sb, in_=bk_d)
            bv_sb = cp.tile([1, GD], f32, tag="bv")
            bo_sb = cp.tile([1, D], f32, tag="bo")
            nc.sync.dma_start(out=bv_sb, in_=bv_d)
            nc.sync.dma_start(out=bo_sb, in_=bo_d)
            id_sb = cp.tile([128, 128], bfl, tag="id")
            of_sb = cp.tile([1, 128], f32, tag="of")
            nc.sync.dma_start(out=id_sb, in_=id_d)
            nc.sync.dma_start(out=of_sb, in_=of_d)
            if causal:
                md_sb = cp.tile([128, NT, 128], bfl, tag="md")
                nc.sync.dma_start(out=md_sb, in_=md_d.rearrange("j p k -> p j k"))

            QT_sb = pr.tile([128, 2, T], bfl, tag="qt")
            KT_sb = pr.tile([128, 2, T], bfl, tag="kt")
            V_sb = pr.tile([128, NT, GH, HD + 1], bfl, tag="v")
            Ocat_sb = pr.tile([128, 2, T], bfl, tag="ocat")

            # ones column of V_aug (softmax denominator accumulator)
            for h in range(GH):
                nc.vector.memset(V_sb[:, :, h, HD:HD + 1], 1.0)

            # ---- phase 1: Q^T, K^T, V projections ----
            for m in range(2):
                for c in range(NQC):
                    qps = sp.tile([128, 512], f32, tag="s")
                    for k in range(KD):
                        nc.tensor.matmul(qps, wq_sb[:, k, ts(m, 128)],
                                         xT_sb[:, k, ts(c, 512)],
                                         start=(k == 0), stop=(k == KD - 1))
                    nc.scalar.activation(QT_sb[:, m, ts(c, 512)], qps, Ident,
                                         bias=bq_sb[:, m:m + 1], scale=SCALE)
                    kps = sp.tile([128, 512], f32, tag="s")
                    for k in range(KD):
                        nc.tensor.matmul(kps, wk_sb[:, k, ts(m, 128)],
                                         xT_sb[:, k, ts(c, 512)],
                                         start=(k == 0), stop=(k == KD - 1))
                    nc.scalar.activation(KT_sb[:, m, ts(c, 512)], kps, Ident,
                                         bias=bk_sb[:, m:m + 1], scale=1.0)
            for tt in range(NT):
                vps = sp.tile([128, 512], f32, tag="s")
                for k in range(KD):
                    nc.tensor.matmul(vps[:, 0:GD], xT_sb[:, k, ts(tt, 128)],
                                     wv_sb[:, k, :],
                                     start=(k == 0), stop=False)
                nc.tensor.matmul(vps[:, 0:GD], of_sb, bv_sb,
                                 start=False, stop=True)
                nc.vector.tensor_copy(
                    V_sb[:, tt, :, 0:HD],
                    vps[:, 0:GD].rearrange("p (h e) -> p h e", h=GH))

            # ---- phase 2: attention ----
            for qc in range(NQC):
                n_kt = 4 * (qc + 1) if causal else NT
                if not causal:
                    mch = mchp.tile([128, NT, 512], bfl, tag="mch")
                    nc.sync.dma_start(
                        out=mch,
                        in_=mt_d.rearrange("(kt p) q -> p kt q", p=128)
                        [:, :, ts(qc, 512)])
                for p in range(2):
                    oA = op.tile([128, 512], f32, tag="o")
                    oB = op.tile([128, 512], f32, tag="o")
                    for kt in range(n_kt):
                        d = kt - 4 * qc
                        diag = causal and d >= 0
                        off = 128 * d if diag else 0
                        w = 512 - off
                        sA = sp.tile([128, 512], f32, tag="s")
                        sB = sp.tile([128, 512], f32, tag="s")
                        qsl = ds(qc * 512 + off, w)
                        last_qk = causal and not diag
                        nc.tensor.matmul(sA[:, off:512],
                                         KT_sb[0:64, p, ts(kt, 128)],
                                         QT_sb[0:64, p, qsl],
                                         start=True, stop=last_qk)
                        nc.tensor.matmul(sB[:, off:512],
                                         KT_sb[64:128, p, ts(kt, 128)],
                                         QT_sb[64:128, p, qsl],
                                         start=True, stop=last_qk)
                        if diag:
                            nc.tensor.matmul(sA[:, off:off + 128],
                                             md_sb[:, kt, :], id_sb,
                                             start=False, stop=True)
                            nc.tensor.matmul(sB[:, off:off + 128],
                                             md_sb[:, kt, :], id_sb,
                                             start=False, stop=True)
                        elif not causal:
                            nc.tensor.matmul(sA, id_sb, mch[:, kt, :],
                                             start=False, stop=True)
                            nc.tensor.matmul(sB, id_sb, mch[:, kt, :],
                                             start=False, stop=True)
                        pA = pp.tile([128, 512], bfl, tag="p")
                        pB = pp.tile([128, 512], bfl, tag="p")
                        if off:
                            nc.vector.memset(pA[:, 0:off], 0.0)
                            nc.vector.memset(pB[:, 0:off], 0.0)
                        nc.scalar.activation(pA[:, off:512], sA[:, off:512], Exp)
                        nc.scalar.activation(pB[:, off:512], sB[:, off:512], Exp)
                        nc.tensor.matmul(oA[0:65, :], V_sb[:, kt, 2 * p, :],
                                         pA, start=(kt == 0),
                                         stop=(kt == n_kt - 1))
                        nc.tensor.matmul(oB[0:65, :], V_sb[:, kt, 2 * p + 1, :],
                                         pB, start=(kt == 0),
                                         stop=(kt == n_kt - 1))
                    # normalize: out rows / denominator (row 64)
                    rA = rp.tile([1, 512], f32, tag="r")
                    rB = rp.tile([1, 512], f32, tag="r")
                    nc.vector.reciprocal(rA, oA[64:65, :])
                    nc.vector.reciprocal(rB, oB[64:65, :])
                    rdA = dr.tile([1, 512], f32, tag="rd")
                    rdB = dr.tile([1, 512], f32, tag="rd")
                    nc.sync.dma_start(out=rdA, in_=rA)
                    nc.sync.dma_start(out=rdB, in_=rB)
                    rbA = rbp.tile([64, 512], f32, tag="rb")
                    rbB = rbp.tile([64, 512], f32, tag="rb")
                    nc.sync.dma_start(out=rbA, in_=rdA.to_broadcast([64, 512]))
                    nc.sync.dma_start(out=rbB, in_=rdB.to_broadcast([64, 512]))
                    nc.vector.tensor_mul(Ocat_sb[0:64, p, ts(qc, 512)],
                                         oA[0:64, :], rbA)
                    obs = obp.tile([64, 512], bfl, tag="obs")
                    nc.vector.tensor_mul(obs, oB[0:64, :], rbB)
                    nc.sync.dma_start(out=Ocat_sb[64:128, p, ts(qc, 512)],
                                      in_=obs)

            # ---- phase 3: output projection (partial over this head group) ----
            for tt in range(NT):
                for ncn in range(2):
                    ops_ = sp.tile([128, 512], f32, tag="s")
                    nc.tensor.matmul(ops_, Ocat_sb[:, 0, ts(tt, 128)],
                                     wo_sb[:, 0, ts(ncn, 512)],
                                     start=True, stop=False)
                    nc.tensor.matmul(ops_, Ocat_sb[:, 1, ts(tt, 128)],
                                     wo_sb[:, 1, ts(ncn, 512)],
                                     start=False, stop=False)
                    nc.tensor.matmul(ops_, of_sb, bo_sb[:, ts(ncn, 512)],
                                     start=False, stop=True)
                    osb = outp.tile([128, 512], f32, tag="ot")
                    nc.vector.tensor_copy(osb, ops_)
                    nc.sync.dma_start(out=out_d[ts(tt, 128), ts(ncn, 512)],
                                      in_=osb)

    nc.compile()
    return nc


def _is_causal_like(m2):
    nb = T // 128
    blk = m2.reshape(nb, 128, nb, 128)
    for j in range(nb):
        for i in range(nb):
            if i < j:
                if np.any(blk[j, :, i, :] != 0.0):
                    return False
            elif i > j:
                if not np.all(blk[j, :, i, :] <= -1e4):
                    return False
    return True


def kernel(x, mask, Wq, bq, Wk, bk, Wv, bv, Wo, bo):
    global LAST_RESULT
    from concourse.bass_utils import run_bass_kernel_spmd

    x = np.asarray(x, dtype=np.float32)
    m2 = np.asarray(mask, dtype=np.float32).reshape(T, T)
    Wq, Wk, Wv, Wo = (np.asarray(w, dtype=np.float32) for w in (Wq, Wk, Wv, Wo))
    bq, bk, bv, bo = (np.asarray(v, dtype=np.float32) for v in (bq, bk, bv, bo))

    causal = _is_causal_like(m2)
    if causal not in _cache:
        _cache[causal] = _build(causal)
    nc = _cache[causal]

    ident = np.eye(128, dtype=bf16)
    onesf = np.ones((1, 128), np.float32)
    if causal:
        maskdiag = np.stack([m2[j * 128:(j + 1) * 128, j * 128:(j + 1) * 128]
                             for j in range(NT)]).astype(bf16)
    else:
        maskT = np.ascontiguousarray(m2.T).astype(bf16)

    xTb = [x[b].T.astype(bf16) for b in range(B)]
    in_maps = []
    for c in range(NCORES):
        b, g = divmod(c, 4)
        sl = slice(g * GD, (g + 1) * GD)
        im = {
            "xT": xTb[b],
            "wq": Wq[:, sl].astype(bf16),
            "wk": Wk[:, sl].astype(bf16),
            "wv": Wv[:, sl].astype(bf16),
            "wo": Wo[sl, :].astype(bf16),
            "bq": np.ascontiguousarray((bq[sl] * SCALE).reshape(2, 128).T),
            "bk": np.ascontiguousarray(bk[sl].reshape(2, 128).T),
            "bv": bv[sl].reshape(1, GD).copy(),
            "bo": (bo if g == 0 else np.zeros_like(bo)).reshape(1, D).copy(),
            "ident": ident,
            "onesf": onesf,
        }
        if causal:
            im["maskdiag"] = maskdiag
        else:
            im["maskT"] = maskT
        in_maps.append(im)

    res = run_bass_kernel_spmd(nc, in_maps, core_ids=list(range(NCORES)),
                               trace=TRACE)
    LAST_RESULT = res

    out = np.empty((B, T, D), np.float32)
    for b in range(B):
        acc = res.results[b * 4 + 0]["out"].copy()
        for g in range(1, 4):
            acc += res.results[b * 4 + g]["out"]
        out[b] = acc
    return out


# revision 3
# speedup vs baseline: 1.6567x; 1.6567x over previous
"""Causal self-attention (B=2, T=2048, D=1024, H=16) on 8 Trainium2 cores.

Sharding: tensor-parallel — core c = (b, g) with b = c // 4 (batch) and
g = c % 4 (head-group of 4 heads / 256 of the 1024 QKV output dims).
Each core computes its head-group's Q/K/V projections, attention, and the
partial output projection (rows g*256:(g+1)*256 of Wo); the host sums the
4 partials per batch (tensor-parallel unshard).

On-chip formulation is fully transposed (scores kept as S^T[k, q]) so no
on-device transposes are needed: the host feeds x^T per batch, and
  Q^T = Wq_g^T · x^T   (lhsT = Wq_g, rhs = x^T)
  S^T = K^T_h^T · Q^T  (lhsT = K^T tile, rhs = Q^T; heads packed in
                        partition halves 0:64 / 64:128 of the dq tiles)
  O^T = V_aug^T · P^T  (lhsT = V with a ones column -> row 64 of the
                        PSUM output accumulates the softmax denominators)
Softmax skips the max-subtraction (scores are O(10) for this problem's
scaling; exp is computed in fp32 from PSUM). The additive mask is applied
exactly: diagonal 128x128 blocks are added via PE matmuls
(lhsT = mask block, rhs = I). Strictly-lower-triangular blocks need no add
and upper blocks are skipped entirely (their exp underflows to 0) — that
fast path is only used when the host verifies the mask has that structure;
otherwise a general variant adds the full mask^T to every score block.

Pipelining: per (q-chunk, head-pair) the AV matmuls lag the QK matmuls by
one k-tile so the TensorE stream never waits on ScalarE's exp; both heads
of a pair share one 2-bank score PSUM tile and a single exp ACTIVATE.
"""

import numpy as np
import ml_dtypes

bf16 = ml_dtypes.bfloat16

B, T, D = 2, 2048, 1024
H, HD = 16, 64
NCORES = 8
GH = 4                  # heads per core
GD = GH * HD            # 256 per-core qkv dims
NT = T // 128           # 16 t-tiles
KD = D // 128           # 8 contraction tiles over D
NQC = T // 512          # 4 q-chunks
SCALE = HD ** -0.5

TRACE = False
LAST_RESULT = None
_cache = {}


def _build(causal):
    import concourse.mybir as mybir
    import concourse.tile as tile
    from concourse import bacc
    from concourse.bass import ds, ts

    f32 = mybir.dt.float32
    bfl = mybir.dt.bfloat16
    Exp = mybir.ActivationFunctionType.Exp
    Ident = mybir.ActivationFunctionType.Identity

    nc = bacc.Bacc("TRN2", target_bir_lowering=False, debug=False,
                   num_devices=NCORES)

    xT_d = nc.dram_tensor("xT", [D, T], bfl, kind="ExternalInput").ap()
    wq_d = nc.dram_tensor("wq", [D, GD], bfl, kind="ExternalInput").ap()
    wk_d = nc.dram_tensor("wk", [D, GD], bfl, kind="ExternalInput").ap()
    wv_d = nc.dram_tensor("wv", [D, GD], bfl, kind="ExternalInput").ap()
    wo_d = nc.dram_tensor("wo", [GD, D], bfl, kind="ExternalInput").ap()
    bq_d = nc.dram_tensor("bq", [128, 2], f32, kind="ExternalInput").ap()
    bk_d = nc.dram_tensor("bk", [128, 2], f32, kind="ExternalInput").ap()
    bv_d = nc.dram_tensor("bv", [1, GD], f32, kind="ExternalInput").ap()
    bo_d = nc.dram_tensor("bo", [1, D], f32, kind="ExternalInput").ap()
    id_d = nc.dram_tensor("ident", [128, 128], bfl, kind="ExternalInput").ap()
    if causal:
        md_d = nc.dram_tensor("maskdiag", [NT, 128, 128], bfl,
                              kind="ExternalInput").ap()
    else:
        mt_d = nc.dram_tensor("maskT", [T, T], bfl, kind="ExternalInput").ap()
    out_d = nc.dram_tensor("out", [T, D], f32, kind="ExternalOutput").ap()

    with tile.TileContext(nc) as tc:
        with tc.tile_pool(name="cp", bufs=1) as cp, \
             tc.tile_pool(name="pr", bufs=1) as pr, \
             tc.tile_pool(name="pp", bufs=3) as pp, \
             tc.tile_pool(name="rp", bufs=4) as rp, \
             tc.tile_pool(name="rbp", bufs=4) as rbp, \
             tc.tile_pool(name="obp", bufs=3) as obp, \
             tc.tile_pool(name="outp", bufs=3) as outp, \
             tc.tile_pool(name="mchp", bufs=2) as mchp, \
             tc.tile_pool(name="sp", bufs=2, space="PSUM") as sp, \
             tc.tile_pool(name="op", bufs=4, space="PSUM") as op, \
             tc.tile_pool(name="dr", bufs=4, space="DRAM") as dr:

            # ---- constant loads ----
            xT_sb = cp.tile([128, KD, T], bfl, tag="xt")
            nc.sync.dma_start(out=xT_sb, in_=xT_d.rearrange("(k p) t -> p k t", p=128))
            wq_sb = cp.tile([128, KD, GD], bfl, tag="wq")
            wk_sb = cp.tile([128, KD, GD], bfl, tag="wk")
            wv_sb = cp.tile([128, KD, GD], bfl, tag="wv")
            nc.sync.dma_start(out=wq_sb, in_=wq_d.rearrange("(k p) m -> p k m", p=128))
            nc.sync.dma_start(out=wk_sb, in_=wk_d.rearrange("(k p) m -> p k m", p=128))
            nc.sync.dma_start(out=wv_sb, in_=wv_d.rearrange("(k p) m -> p k m", p=128))
            wo_sb = cp.tile([128, 2, D], bfl, tag="wo")
            nc.sync.dma_start(out=wo_sb, in_=wo_d.rearrange("(m p) n -> p m n", p=128))
            bq_sb = cp.tile([128, 2], f32, tag="bq")
            bk_sb = cp.tile([128, 2], f32, tag="bk")
            nc.sync.dma_start(out=bq_sb, in_=bq_d)
            nc.sync.dma_start(out=bk_sb, in_=bk_d)
            # biases along the free dim: broadcast across partitions once
            bv_bc = cp.tile([128, GD], f32, tag="bvb")
            bo_bc = cp.tile([128, D], f32, tag="bob")
            nc.sync.dma_start(out=bv_bc, in_=bv_d.to_broadcast([128, GD]))
            nc.sync.dma_start(out=bo_bc, in_=bo_d.to_broadcast([128, D]))
            id_sb = cp.tile([128, 128], bfl, tag="id")
            nc.sync.dma_start(out=id_sb, in_=id_d)
            if causal:
                md_sb = cp.tile([128, NT, 128], bfl, tag="md")
                nc.sync.dma_start(out=md_sb, in_=md_d.rearrange("j p k -> p j k"))

            QT_sb = pr.tile([128, 2, T], bfl, tag="qt")
            KT_sb = pr.tile([128, 2, T], bfl, tag="kt")
            V_sb = pr.tile([128, NT, GH, HD + 1], bfl, tag="v")
            Ocat_sb = pr.tile([128, 2, T], bfl, tag="ocat")

            # ones column of V_aug (softmax denominator accumulator)
            for h in range(GH):
                nc.vector.memset(V_sb[:, :, h, HD:HD + 1], 1.0)

            # ---- phase 1: Q^T, K^T, V projections ----
            for m in range(2):
                for c in range(NQC):
                    qps = sp.tile([128, 2, 512], f32, tag="s")
                    for k in range(KD):
                        nc.tensor.matmul(qps[:, 0, :], wq_sb[:, k, ts(m, 128)],
                                         xT_sb[:, k, ts(c, 512)],
                                         start=(k == 0), stop=(k == KD - 1))
                    for k in range(KD):
                        nc.tensor.matmul(qps[:, 1, :], wk_sb[:, k, ts(m, 128)],
                                         xT_sb[:, k, ts(c, 512)],
                                         start=(k == 0), stop=(k == KD - 1))
                    nc.scalar.activation(QT_sb[:, m, ts(c, 512)], qps[:, 0, :],
                                         Ident, bias=bq_sb[:, m:m + 1],
                                         scale=SCALE)
                    nc.scalar.activation(KT_sb[:, m, ts(c, 512)], qps[:, 1, :],
                                         Ident, bias=bk_sb[:, m:m + 1],
                                         scale=1.0)
            for tt in range(NT):
                vps = sp.tile([128, 2, 512], f32, tag="s")
                for k in range(KD):
                    nc.tensor.matmul(vps[:, 0, 0:GD], xT_sb[:, k, ts(tt, 128)],
                                     wv_sb[:, k, :],
                                     start=(k == 0), stop=(k == KD - 1))
                nc.vector.tensor_add(
                    V_sb[:, tt, :, 0:HD],
                    vps[:, 0, 0:GD].rearrange("p (h e) -> p h e", h=GH),
                    bv_bc.rearrange("p (h e) -> p h e", h=GH))

            # ---- phase 2: attention ----
            for qc in range(NQC):
                n_kt = 4 * (qc + 1) if causal else NT
                if not causal:
                    mch = mchp.tile([128, NT, 512], bfl, tag="mch")
                    nc.sync.dma_start(
                        out=mch,
                        in_=mt_d.rearrange("(kt p) q -> p kt q", p=128)
                        [:, :, ts(qc, 512)])
                for p in range(2):
                    oA = op.tile([128, 512], f32, tag="o")
                    oB = op.tile([128, 512], f32, tag="o")
                    plist = [None] * n_kt
                    offs = [0] * n_kt
                    # AV lags QK by one k-tile: PE never waits on the exp
                    for kt in range(n_kt + 1):
                        if kt < n_kt:
                            d = kt - 4 * qc
                            diag = causal and d >= 0
                            off = 128 * d if diag else 0
                            offs[kt] = off
                            w = 512 - off
                            s2 = sp.tile([128, 2, 512], f32, tag="s")
                            qsl = ds(qc * 512 + off, w)
                            last_qk = causal and not diag
                            nc.tensor.matmul(s2[:, 0, off:512],
                                             KT_sb[0:64, p, ts(kt, 128)],
                                             QT_sb[0:64, p, qsl],
                                             start=True, stop=last_qk)
                            nc.tensor.matmul(s2[:, 1, off:512],
                                             KT_sb[64:128, p, ts(kt, 128)],
                                             QT_sb[64:128, p, qsl],
                                             start=True, stop=last_qk)
                            if diag:
                                nc.tensor.matmul(s2[:, 0, off:off + 128],
                                                 md_sb[:, kt, :], id_sb,
                                                 start=False, stop=True)
                                nc.tensor.matmul(s2[:, 1, off:off + 128],
                                                 md_sb[:, kt, :], id_sb,
                                                 start=False, stop=True)
                            elif not causal:
                                nc.tensor.matmul(s2[:, 0, :], id_sb,
                                                 mch[:, kt, :],
                                                 start=False, stop=True)
                                nc.tensor.matmul(s2[:, 1, :], id_sb,
                                                 mch[:, kt, :],
                                                 start=False, stop=True)
                            p2 = pp.tile([128, 2, 512], bfl, tag="p")
                            plist[kt] = p2
                            if off:
                                nc.vector.memset(p2[:, :, 0:off], 0.0)
                            nc.scalar.activation(p2[:, :, off:512],
                                                 s2[:, :, off:512], Exp)
                        if kt >= 1:
                            pk = plist[kt - 1]
                            nc.tensor.matmul(oA[0:65, :],
                                             V_sb[:, kt - 1, 2 * p, :],
                                             pk[:, 0, :], start=(kt == 1),
                                             stop=(kt == n_kt))
                            nc.tensor.matmul(oB[0:65, :],
                                             V_sb[:, kt - 1, 2 * p + 1, :],
                                             pk[:, 1, :], start=(kt == 1),
                                             stop=(kt == n_kt))
                    # normalize: out rows / denominator (row 64).
                    # reciprocal_approx_fast (custom DVE op) requires base
                    # partition 0 — compute over the whole [0:65] block and
                    # use only row 64 (other lanes are don't-care).
                    rA = rp.tile([65, 512], f32, tag="r")
                    rB = rp.tile([65, 512], f32, tag="r")
                    nc.vector.reciprocal_approx_fast(out=rA, in_=oA[0:65, :])
                    nc.vector.reciprocal_approx_fast(out=rB, in_=oB[0:65, :])
                    rdA = dr.tile([1, 512], f32, tag="rd")
                    rdB = dr.tile([1, 512], f32, tag="rd")
                    nc.sync.dma_start(out=rdA, in_=rA[64:65, :])
                    nc.sync.dma_start(out=rdB, in_=rB[64:65, :])
                    rbA = rbp.tile([64, 512], f32, tag="rb")
                    rbB = rbp.tile([64, 512], f32, tag="rb")
                    nc.sync.dma_start(out=rbA, in_=rdA.to_broadcast([64, 512]))
                    nc.sync.dma_start(out=rbB, in_=rdB.to_broadcast([64, 512]))
                    nc.vector.tensor_mul(Ocat_sb[0:64, p, ts(qc, 512)],
                                         oA[0:64, :], rbA)
                    obs = obp.tile([64, 512], bfl, tag="obs")
                    nc.vector.tensor_mul(obs, oB[0:64, :], rbB)
                    nc.sync.dma_start(out=Ocat_sb[64:128, p, ts(qc, 512)],
                                      in_=obs)

            # ---- phase 3: output projection (partial over this head group) ----
            for tt in range(NT):
                for ncn in range(2):
                    ops_ = sp.tile([128, 2, 512], f32, tag="s")
                    nc.tensor.matmul(ops_[:, 0, :], Ocat_sb[:, 0, ts(tt, 128)],
                                     wo_sb[:, 0, ts(ncn, 512)],
                                     start=True, stop=False)
                    nc.tensor.matmul(ops_[:, 0, :], Ocat_sb[:, 1, ts(tt, 128)],
                                     wo_sb[:, 1, ts(ncn, 512)],
                                     start=False, stop=True)
                    osb = outp.tile([128, 512], f32, tag="ot")
                    nc.vector.tensor_add(osb, ops_[:, 0, :],
                                         bo_bc[:, ts(ncn, 512)])
                    nc.sync.dma_start(out=out_d[ts(tt, 128), ts(ncn, 512)],
                                      in_=osb)

    nc.compile()
    return nc


def _is_causal_like(m2):
    nb = T // 128
    blk = m2.reshape(nb, 128, nb, 128)
    for j in range(nb):
        for i in range(nb):
            if i < j:
                if np.any(blk[j, :, i, :] != 0.0):
                    return False
            elif i > j:
                if not np.all(blk[j, :, i, :] <= -1e4):
                    return False
    return True


def kernel(x, mask, Wq, bq, Wk, bk, Wv, bv, Wo, bo):
    global LAST_RESULT
    from concourse.bass_utils import run_bass_kernel_spmd

    x = np.asarray(x, dtype=np.float32)
    m2 = np.asarray(mask, dtype=np.float32).reshape(T, T)
    Wq, Wk, Wv, Wo = (np.asarray(w, dtype=np.float32) for w in (Wq, Wk, Wv, Wo))
    bq, bk, bv, bo = (np.asarray(v, dtype=np.float32) for v in (bq, bk, bv, bo))

    causal = _is_causal_like(m2)
    if causal not in _cache:
        _cache[causal] = _build(causal)
    nc = _cache[causal]

    ident = np.eye(128, dtype=bf16)
    if causal:
        maskdiag = np.stack([m2[j * 128:(j + 1) * 128, j * 128:(j + 1) * 128]
                             for j in range(NT)]).astype(bf16)
    else:
        maskT = np.ascontiguousarray(m2.T).astype(bf16)

    xTb = [x[b].T.astype(bf16) for b in range(B)]
    in_maps = []
    for c in range(NCORES):
        b, g = divmod(c, 4)
        sl = slice(g * GD, (g + 1) * GD)
        im = {
            "xT": xTb[b],
            "wq": Wq[:, sl].astype(bf16),
            "wk": Wk[:, sl].astype(bf16),
            "wv": Wv[:, sl].astype(bf16),
            "wo": Wo[sl, :].astype(bf16),
            "bq": np.ascontiguousarray((bq[sl] * SCALE).reshape(2, 128).T),
            "bk": np.ascontiguousarray(bk[sl].reshape(2, 128).T),
            "bv": bv[sl].reshape(1, GD).copy(),
            "bo": (bo if g == 0 else np.zeros_like(bo)).reshape(1, D).copy(),
            "ident": ident,
        }
        if causal:
            im["maskdiag"] = maskdiag
        else:
            im["maskT"] = maskT
        in_maps.append(im)

    res = run_bass_kernel_spmd(nc, in_maps, core_ids=list(range(NCORES)),
                               trace=TRACE)
    LAST_RESULT = res

    out = np.empty((B, T, D), np.float32)
    for b in range(B):
        acc = res.results[b * 4 + 0]["out"].copy()
        for g in range(1, 4):
            acc += res.results[b * 4 + g]["out"]
        out[b] = acc
    return out
